# revision 74
# baseline (speedup 1.0000x reference)
"""Causal self-attention (dense transformer) on 8 TRN2 NeuronCores.

Sharding: heads+batch tensor-parallel. Each core c owns 2 heads (2c, 2c+1)
for all 4 batches:
  - QKV projection in 3-term compensated fp8 (x8*w8 + xlo*w8 + x8*wlo, each
    DoubleRow-paired over two 128-d contraction tiles) — near-bf16 accuracy
    at 4x the per-instruction throughput. q,k copied out in bf16, v in
    fp8 hi + fp8 residual (v8 + vlo).
  - Causal attention in "transposed scores" layout S_t[tk, tq]: QK^T in
    bf16, exp on the Act engine writes probabilities directly in fp8 into
    kt-pair tiles, PV runs fp8 DoubleRow over kt pairs (v8 and vlo terms).
    Diagonal tiles are masked post-exp on DVE: tri for the even tile of a
    pair, zeros|tri for the odd one (also covering its never-written band
    so the pair-shared DoubleRow range stays valid). The v stationary is
    96 cols/head (v|ones|zero-pad; DR ldweights needs multiple-of-32
    group widths and 4-aligned strides to pass walrus ISA checks); the
    ones column yields softmax denominators in psum row 64.
  - AllToAll exchanges y feature-slices for token-slices in groups
    (b0+b1 together, b2, b3), issued 2 units after their data completes
    so staging DMAs never block Pool's in-order queue; each core then
    projects its token chunks (bf16).
Host side: quantize/shard/transpose inputs, concat+transpose the output.
"""

import numpy as np
import ml_dtypes

import concourse.bass as bass
import concourse.mybir as mybir
import concourse.tile as tile
from concourse.bass_utils import run_bass_kernel_spmd

BF16 = mybir.dt.bfloat16
F32 = mybir.dt.float32
FP8 = mybir.dt.float8e4
AF = mybir.ActivationFunctionType
DR = mybir.MatmulPerfMode.DoubleRow

# Full-size problem constants (hardcoded per harness contract)
N_CORES = 8
BSZ, SEQ, D, N_HEAD = 4, 2048, 1024, 16
HD = 64  # head dim
W_SCALE = 32.0  # w_qkv pre-scale so fp8 hi/lo quantization is well-ranged


def _split_multi_waits(nc):
    """walrus on this build accepts at most ONE sync-wait command per
    instruction. Hoist extra waits onto standalone same-engine nops placed
    immediately before the instruction (queue order preserves semantics)."""
    edits = []
    for func in nc.m.functions:
        for bb in func.blocks:
            insts = bb.instructions
            for idx, ins in enumerate(insts):
                si = ins.sync_info
                if si is not None and len(si.on_wait) > 1:
                    edits.append((bb, idx, ins))
    for bb, idx, ins in reversed(edits):
        si = ins.sync_info
        extra, keep = list(si.on_wait[:-1]), [si.on_wait[-1]]
        ins.sync_info = mybir.SyncInfo(on_wait=keep, on_update=list(si.on_update))
        nops = []
        for w in extra:
            nop = nc.engines[ins.engine].nop().ins
            host = nc.cur_bb.bb.instructions
            assert host[-1] is nop
            host.pop()
            nop.sync_info = mybir.SyncInfo(on_wait=[w], on_update=[])
            nops.append(nop)
        live = bb.instructions
        for j, nop in enumerate(nops):
            live.insert(idx + j, nop)


def build_nc(n_cores=N_CORES, bsz=BSZ, seq=SEQ, d=D, n_head=N_HEAD):
    hd = HD
    hpc = n_head // n_cores          # heads per core
    fl = hpc * hd                    # local feature width (q/k/v per core)
    T = bsz * seq                    # total tokens
    kd = d // 128                    # contraction tiles over d
    kp = kd // 2                     # DoubleRow pair steps over d
    tb = min(512, seq)               # tq block width (matmul free dim)
    nqb = seq // tb                  # q-blocks per batch
    dtiles = tb // 128               # 128-tiles per q-block (diag masks)
    nt = T // 128                    # total 128-token tiles
    scale = float(1.0 / np.sqrt(hd))
    inv_w = float(1.0 / W_SCALE)

    tsb = seq // n_cores             # per-batch token chunk per core
    nc = bass.Bass(num_devices=n_cores)
    # x8 then xlo8 stacked: [2, d, T] fp8
    xboth = nc.declare_dram_parameter("xboth", [2, d, T], FP8, isOutput=False)
    # w hi/lo packed for DoubleRow: [kp, 128, 2, 3*fl]
    wq8 = nc.declare_dram_parameter("wq8", [kp, 128, 2, 3 * fl], FP8, isOutput=False)
    wqlo = nc.declare_dram_parameter("wqlo", [kp, 128, 2, 3 * fl], FP8, isOutput=False)
    wproj = nc.declare_dram_parameter("wproj", [d, d], BF16, isOutput=False)
    out = nc.declare_dram_parameter("out", [d, bsz * tsb], F32, isOutput=True)
    # collective groups: batches 0+1 exchange together (one 15us flat
    # overhead instead of two); 2 alone; batch 3 in two halves so the
    # tail collective is small and the first overlaps its own attention
    assert bsz == 4 and nqb == 4
    groups = [(0, 1), (2,), (3,)]
    a2a_in = [nc.dram_tensor(f"a2a_in{g}", [n_cores, fl, len(gr) * tsb], BF16)
              for g, gr in enumerate(groups)]
    a2a_out = [nc.dram_tensor(f"a2a_out{g}", [n_cores, fl, len(gr) * tsb], BF16)
               for g, gr in enumerate(groups)]
    # batch-3 segments (token units within the batch): 3/4 after (3,qb2),
    # final 1/4 at the end so the tail collective is flat-overhead bound
    b3segs = [(0, seq // 2), (seq // 2, seq // 2)]
    a2ah_in = [nc.dram_tensor(f"a2ah_in{h}", [n_cores, fl, ln // n_cores], BF16)
               for h, (st, ln) in enumerate(b3segs)]
    a2ah_out = [nc.dram_tensor(f"a2ah_out{h}", [n_cores, fl, ln // n_cores], BF16)
                for h, (st, ln) in enumerate(b3segs)]

    with tile.TileContext(nc) as tc:
        with (
            tc.tile_pool(name="const", bufs=1) as const,
            tc.tile_pool(name="xin", bufs=2) as xin,
            tc.tile_pool(name="work", bufs=3) as work,
            tc.tile_pool(name="psum", bufs=1, space="PSUM") as psum,
        ):
            # ---- persistent SBUF ----
            w8_sb = [const.tile([128, 2 * 3 * fl], FP8, name=f"w8_sb{i}")
                     for i in range(kp)]
            wlo_sb = [const.tile([128, 2 * 3 * fl], FP8, name=f"wlo_sb{i}")
                      for i in range(kp)]

            wp_sb = [const.tile([fl, d], BF16, name=f"wp_sb{i}") for i in range(n_cores)]

            q_sb = const.tile([fl, T], BF16, name="q_sb")
            k_sb = const.tile([fl, T], BF16, name="k_sb")
            y_sb = const.tile([fl, T], BF16, name="y_sb")
            # v token-major, hi and lo fp8 parts; per (tile, head) 96 cols:
            # [v 0:64 | ones col 64 (v8; zeros in vlo) -> softmax denom |
            #  zero pad 65:96]. 96 because DoubleRow ldweights requires the
            # stationary free-per-group to be a multiple of 32 (and strides
            # 4-aligned) to pass walrus ISA checks.
            vwh = 96
            vw = hpc * vwh
            v8_sb = const.tile([128, nt * vw], FP8, name="v8_sb")
            vlo_sb = const.tile([128, nt * vw], FP8, name="vlo_sb")
            ones8 = v8_sb.rearrange("p (n h c) -> p n h c", h=hpc,
                                    c=vwh)[:, :, :, hd:hd + 1]
            nc.gpsimd.memset(ones8, 1.0)
            pad8 = v8_sb.rearrange("p (n h c) -> p n h c", h=hpc,
                                   c=vwh)[:, :, :, hd + 1:vwh]
            nc.gpsimd.memset(pad8, 0.0)
            lo_tail = vlo_sb.rearrange("p (n h c) -> p n h c", h=hpc,
                                       c=vwh)[:, :, :, hd:vwh]
            nc.gpsimd.memset(lo_tail, 0.0)

            ones_col = const.tile([1, hd], BF16, name="ones_col")
            nc.vector.memset(ones_col[:], 1.0)

            # per-partition exp bias (see attn_block)
            expbias = const.tile([128, 1], F32, name="expbias")
            nc.vector.memset(expbias[:], -3.0)

            # triangular mask [128,128] fp8: keep S_t[tk_i, tq_j] iff i <= j
            tri = const.tile([128, 128], FP8, name="tri")
            nc.gpsimd.memset(tri[:], 1.0)
            nc.gpsimd.affine_select(
                out=tri[:], in_=tri[:],
                compare_op=mybir.AluOpType.is_ge, fill=0.0,
                base=0, channel_multiplier=-1, pattern=[[1, 128]],
            )
            # zeros|triangle mask [128,256]: col j keeps row p iff p <= j-128.
            # Used on the second diag tile of a kt pair: zeroes the 128-col
            # band below its diagonal (which exp never writes) and masks its
            # triangle, so the pair-shared DoubleRow PV range is valid.
            zt = const.tile([128, 256], FP8, name="zt")
            nc.gpsimd.memset(zt[:], 1.0)
            nc.gpsimd.affine_select(
                out=zt[:], in_=zt[:],
                compare_op=mybir.AluOpType.is_ge, fill=0.0,
                base=-128, channel_multiplier=-1, pattern=[[1, 256]],
            )
            # pre-zero the 3 rotating p_pair slots so first-use stale bytes
            # can never inject NaN through the mask multiply
            for _ in range(7):
                tz = work.tile([128, 2 * hpc * tb], FP8, name="p_pair",
                               tag="pt", bufs=7)
                nc.gpsimd.memset(tz[:], 0.0)

            # ---- unified (batch, q-block) stream with qkv prefetch ----
            def fetch_x(b, qb):
                tbi = b * (seq // tb) + qb
                ts0 = tbi * tb
                # one DMA: both x variants, all d-tiles for this token block
                x_t = xin.tile([128, 2 * kd * tb], FP8, name="x_t", tag="x", bufs=3)
                nc.sync.dma_start(
                    x_t[:].rearrange("p (v i t) -> p v i t", v=2, i=kd),
                    xboth.rearrange("v (i p) T -> p v i T", p=128)[:, :, :, ts0:ts0 + tb])
                return x_t

            def qkv_block(b, qb, pre=None):
                tbi = b * (seq // tb) + qb
                ts0 = tbi * tb
                x_t = pre if pre is not None else fetch_x(b, qb)
                xv = x_t[:].rearrange("p (v i t) -> p v i t", v=2, i=kd)
                # q, k (feature-major): out [fl, tb], 3 fp8 terms x kp pairs
                for which, dst in ((0, q_sb), (1, k_sb)):
                    ps = psum.tile([fl, tb], F32, name=f"ps_qk{which}", tag="mm512", bufs=2)
                    n_mm = 3 * kp
                    mi = 0
                    for wsb, xvi in ((w8_sb, 0), (w8_sb, 1), (wlo_sb, 0)):
                        for j in range(kp):
                            lhs = wsb[j][:].rearrange("p (g m) -> p g m", g=2)[
                                :, :, which * fl:(which + 1) * fl]
                            nc.tensor.matmul(
                                ps[:], lhs, xv[:, xvi, 2 * j:2 * j + 2, :],
                                start=(mi == 0), stop=(mi == n_mm - 1),
                                perf_mode=DR)
                            mi += 1
                    with nc.allow_low_precision("qkv copies"):
                        nc.vector.tensor_scalar_mul(dst[:, ts0:ts0 + tb], ps[:], inv_w)
                # v (token-major): out [128 tok, fl] -> v8 + vlo fp8
                for tt in range(dtiles):
                    gti = tbi * dtiles + tt
                    ps_v = psum.tile([128, fl], F32, name="ps_v", tag="mm512", bufs=2)
                    n_mm = 3 * kp
                    mi = 0
                    for wsb, xvi in ((w8_sb, 0), (w8_sb, 1), (wlo_sb, 0)):
                        for j in range(kp):
                            lhs = xv[:, xvi, 2 * j:2 * j + 2, tt * 128:(tt + 1) * 128]
                            rhs = wsb[j][:].rearrange("p (g m) -> p g m", g=2)[
                                :, :, 2 * fl:3 * fl]
                            nc.tensor.matmul(
                                ps_v[:], lhs, rhs,
                                start=(mi == 0), stop=(mi == n_mm - 1),
                                perf_mode=DR)
                            mi += 1
                    v8_ap = v8_sb.rearrange("p (n h c) -> p n h c", h=hpc,
                                            c=vwh)[:, gti, :, 0:hd]
                    vlo_ap = vlo_sb.rearrange("p (n h c) -> p n h c", h=hpc,
                                              c=vwh)[:, gti, :, 0:hd]
                    psh = ps_v[:].rearrange("p (h c) -> p h c", c=hd)
                    with nc.allow_low_precision("v fp8 split"):
                        nc.vector.tensor_scalar_mul(v8_ap, psh, inv_w)
                        nc.vector.scalar_tensor_tensor(
                            vlo_ap, psh, inv_w, v8_ap,
                            op0=mybir.AluOpType.mult,
                            op1=mybir.AluOpType.subtract)

            def v_pair_ap(vsb, gti0, ngrp, h):
                """[128, ngrp, 96] AP over v tiles gti0..gti0+ngrp-1, head h."""
                return vsb.rearrange("p (n h c) -> p n h c", h=hpc,
                                     c=vwh)[:, gti0:gti0 + ngrp, h, :]

            def attn_block(b, qb, norm_prev=None):
                tq0 = b * seq + qb * tb
                ntk = (qb + 1) * dtiles
                ps_y = [psum.tile([96, tb], F32, name=f"ps_y{h}",
                                  tag=f"y{h}", bufs=1) for h in range(hpc)]
                pv_issued = [0] * hpc
                # total PV matmuls per head: 2 terms x (all kt pairs)
                n_pv = 2 * (ntk // 2)
                pv_queue = []

                def issue_pv(gti0, pgv, c0p):
                    for h in range(hpc):
                        for vsb in (v8_sb, vlo_sb):
                            nc.tensor.matmul(
                                ps_y[h][:, c0p:tb],
                                v_pair_ap(vsb, gti0, 2, h),
                                pgv[:, :, h, c0p:tb],
                                start=(pv_issued[h] == 0),
                                stop=(pv_issued[h] == n_pv - 1),
                                perf_mode=DR)
                            pv_issued[h] += 1
                p_pair = None
                for tki in range(ntk):
                    t0 = b * seq + tki * 128
                    gti = t0 // 128
                    m = tki - qb * dtiles
                    c0 = 128 * m if m > 0 else 0
                    diag = m >= 0
                    if tki % 2 == 0:
                        p_pair = work.tile([128, 2 * hpc * tb], FP8, name="p_pair",
                                           tag="pt", bufs=7)
                    pg = p_pair[:].rearrange("p (g h t) -> p g h t", g=2, h=hpc)
                    # scores for both heads into one 2-bank psum tile
                    ps_s = psum.tile([128, hpc * tb], F32, name="ps_s",
                                     tag="s2", bufs=2)
                    for h in range(hpc):
                        hs = slice(h * hd, (h + 1) * hd)
                        nc.tensor.matmul(ps_s[:, h * tb + c0:(h + 1) * tb],
                                         k_sb[hs, t0:t0 + 128],
                                         q_sb[hs, tq0 + c0:tq0 + tb],
                                         start=True, stop=True)
                    # one exp for both heads -> fp8 probabilities in pair tile
                    sv = ps_s[:].rearrange("p (h q) -> p h q", h=hpc)[:, :, c0:tb]
                    pv = pg[:, tki % 2, :, c0:tb]
                    # bias -3 keeps p=exp(s/8-3) under fp8e4m3 max (240);
                    # the ones-column denominator scales identically, so the
                    # softmax ratio is unchanged.
                    with nc.allow_low_precision("p fp8"):
                        nc.scalar.activation(pv, sv, AF.Exp, scale=scale,
                                             bias=expbias[:])
                    if tki == 1 and norm_prev is not None:
                        norm_prev()
                        norm_prev = None
                    if diag:
                        if tki % 2 == 1 and m > 0:
                            # second diag of a pair: zero the unwritten band
                            # [c0-128, c0) and mask the triangle in one op
                            for h in range(hpc):
                                ap = pg[:, tki % 2, h, c0 - 128:c0 + 128]
                                nc.vector.tensor_mul(ap, ap, zt[:])
                        else:
                            # mask the [128,128] triangle at cols [c0, c0+128)
                            for h in range(hpc):
                                ap = pg[:, tki % 2, h, c0:c0 + 128]
                                nc.vector.tensor_mul(ap, ap, tri[:])
                    if tki % 2 == 1:
                        # completed kt pair: DoubleRow PV, both terms, from
                        # the pair's first tile's c0 (c0=0 off-diagonal).
                        # ISSUE LAGGED by one pair: PV waits on exp, and PE
                        # dispatches in order, so a PV issued before the next
                        # QKs head-of-line blocks them and serializes the
                        # whole exp->PV->QK->exp chain. Queueing PV behind
                        # the following pair's QKs keeps PE fed.
                        mp = (tki - 1) - qb * dtiles
                        c0p = 128 * mp if mp > 0 else 0
                        pv_queue.append((gti - 1, pg, c0p))
                        if len(pv_queue) >= 5:
                            issue_pv(*pv_queue.pop(0))
                for item in pv_queue:
                    issue_pv(*item)
                for h in range(hpc):
                    assert pv_issued[h] == n_pv, (pv_issued[h], n_pv)

                # normalize deferred into the NEXT block: the recip/bc/mul
                # chain waits on PV-stop, and issuing it here head-of-line
                # blocks the next block's QKs behind it on PE/DVE
                def do_norm():
                    for h in range(hpc):
                        recipb = work.tile([1, tb], BF16, name="recipb", tag="recipb", bufs=2)
                        with nc.allow_low_precision("softmax 1/denom in bf16"):
                            nc.vector.reciprocal(recipb[:], ps_y[h][hd:hd + 1, :])
                        ps_b = psum.tile([hd, tb], F32, name="ps_b", tag="mm512", bufs=2)
                        nc.tensor.matmul(ps_b[:], ones_col[:], recipb[:],
                                         start=True, stop=True)
                        # DVE can read only ONE psum operand per instruction:
                        # stage the broadcast reciprocal through SBUF first
                        bc_sb = work.tile([hd, tb], BF16, name="bc_sb", tag="bcsb", bufs=2)
                        nc.vector.tensor_copy(bc_sb[:], ps_b[:])
                        nc.vector.tensor_mul(y_sb[h * hd:(h + 1) * hd, tq0:tq0 + tb],
                                             ps_y[h][0:hd, :], bc_sb[:])
                if norm_prev is not None:
                    norm_prev()  # ntk was too short to flush it mid-block
                return do_norm

            def a2a_issue(g):
                # shard j of group g = per batch b in group, tokens
                # b*seq + [j*tsb, (j+1)*tsb), concatenated along columns
                for j in range(n_cores):
                    for bi, b in enumerate(groups[g]):
                        nc.gpsimd.dma_start(
                            a2a_in[g][j][:, bi * tsb:(bi + 1) * tsb],
                            y_sb[:, b * seq + j * tsb:b * seq + (j + 1) * tsb])
                nc.gpsimd.collective_compute(
                    "AllToAll", mybir.AluOpType.bypass,
                    replica_groups=[list(range(n_cores))],
                    ins=[a2a_in[g][:]], outs=[a2a_out[g][:]],
                )

            def a2a_issue_half(h):
                # batch-3 segment h: shard j = tokens 3*seq + st + [j*w, ..)
                st, ln = b3segs[h]
                w = ln // n_cores
                base = 3 * seq + st
                for j in range(n_cores):
                    nc.gpsimd.dma_start(
                        a2ah_in[h][j],
                        y_sb[:, base + j * w:base + (j + 1) * w])
                nc.gpsimd.collective_compute(
                    "AllToAll", mybir.AluOpType.bypass,
                    replica_groups=[list(range(n_cores))],
                    ins=[a2ah_in[h][:]], outs=[a2ah_out[h][:]],
                )

            def proj_cols(y_srcs, w_cols, c0o):
                # project w_cols token columns starting at output col c0o;
                # y_srcs[i] = DRAM slice [fl, w_cols] for core i's features
                y_loc = [work.tile([fl, w_cols], BF16, name="y_loc",
                                   tag=f"yloc{i}", bufs=2) for i in range(n_cores)]
                for i in range(n_cores):
                    nc.sync.dma_start(y_loc[i][:], y_srcs[i])
                for dj in range(d // 128):
                    ps_o = psum.tile([128, w_cols], F32, name="ps_o", tag="mm512", bufs=2)
                    for i in range(n_cores):
                        nc.tensor.matmul(
                            ps_o[:], wp_sb[i][:, dj * 128:(dj + 1) * 128],
                            y_loc[i][:], start=(i == 0), stop=(i == n_cores - 1))
                    o_sb = work.tile([128, w_cols], F32, name="o_sb", tag="osb", bufs=3)
                    nc.vector.tensor_copy(o_sb[:], ps_o[:])
                    nc.sync.dma_start(
                        out[dj * 128:(dj + 1) * 128, c0o:c0o + w_cols], o_sb[:])

            def proj(g, bi, pb):
                proj_cols([a2a_out[g][i][:, bi * tsb:(bi + 1) * tsb]
                           for i in range(n_cores)], tsb, pb * tsb)

            def proj_half(h):
                st, ln = b3segs[h]
                proj_cols([a2ah_out[h][i] for i in range(n_cores)],
                          ln // n_cores, 3 * tsb + st // n_cores)

            units = [(b, qb) for b in range(bsz) for qb in range(nqb)]
            # unit-0's x DMA goes out first; the weight DMAs follow on SP,
            # and only THEN unit-0's matmuls are issued (program order is
            # semantic order for the tile framework — weights must be
            # written before any instruction that reads them is recorded)
            x_t0 = fetch_x(*units[0])
            for i in range(kp):
                nc.sync.dma_start(w8_sb[i][:], wq8[i].rearrange("p g m -> p (g m)"))
            for i in range(kp):
                nc.sync.dma_start(wlo_sb[i][:], wqlo[i].rearrange("p g m -> p (g m)"))
            qkv_block(*units[0], pre=x_t0)
            if len(units) > 1:
                qkv_block(*units[1])
            for i in range(n_cores):
                nc.sync.dma_start(wp_sb[i][:], wproj[i * fl:(i + 1) * fl, :])
            # Collectives are issued 2 units AFTER their data completes so
            # staging DMAs never sit blocked at the head of Pool's in-order
            # queue. Schedule (unit L -> actions):
            #   L9  (b2,qb1): a2a group (0,1)
            #   L13 (b3,qb1): a2a group (2,); a2a b3-half0; proj b0, b1
            #   L14 (b3,qb2): proj b2
            #   end: a2a b3-half1; proj b3-half0; proj b3-half1
            n_units = len(units)
            norm_prev = None
            for L, (b, qb) in enumerate(units):
                norm_prev = attn_block(b, qb, norm_prev)
                if L + 2 < n_units:
                    qkv_block(*units[L + 2])
                if L == 9:
                    a2a_issue(0)
                elif L == 13:
                    # h0 stages THIS block's tokens: flush its deferred norm
                    # first or the staging ships unnormalized y
                    norm_prev()
                    norm_prev = None
                    a2a_issue(1)
                    a2a_issue_half(0)
                    proj(0, 0, 0)
                    proj(0, 1, 1)
            if norm_prev is not None:
                norm_prev()
            a2a_issue_half(1)
            proj_half(0)
            proj(1, 0, 2)
            proj_half(1)
    _split_multi_waits(nc)
    return nc


def shard_inputs(x, w_qkv, w_proj, n_cores=N_CORES, n_head=N_HEAD):
    bf16 = ml_dtypes.bfloat16
    e4 = ml_dtypes.float8_e4m3
    d = x.shape[-1]
    T = x.shape[0] * x.shape[1]
    hpc = n_head // n_cores
    fl = hpc * HD
    kp = d // 256
    xT = np.ascontiguousarray(np.asarray(x, np.float32).reshape(T, d).T)
    x8 = xT.astype(e4)
    xlo = (xT - x8.astype(np.float32)).astype(e4)
    xboth = np.ascontiguousarray(np.stack([x8, xlo], axis=0))
    wq = np.asarray(w_qkv, np.float32)
    wp = np.ascontiguousarray(np.asarray(w_proj, np.float32).T.astype(bf16))
    in_maps = []
    for c in range(n_cores):
        r0 = c * fl
        wqkv_c = np.ascontiguousarray(
            np.concatenate([wq[r0:r0 + fl], wq[d + r0:d + r0 + fl],
                            wq[2 * d + r0:2 * d + r0 + fl]], axis=0).T) * W_SCALE
        w8 = wqkv_c.astype(e4)
        wlo = (wqkv_c - w8.astype(np.float32)).astype(e4)
        # [d, 3fl] -> [kp, 128, 2, 3fl] (pair j, partition, group, col)
        pack = lambda w: np.ascontiguousarray(
            w.reshape(kp, 2, 128, 3 * fl).transpose(0, 2, 1, 3))
        in_maps.append({"xboth": xboth, "wq8": pack(w8), "wqlo": pack(wlo),
                        "wproj": wp})
    return in_maps


def assemble_out(outs, n_cores=N_CORES, bsz=BSZ, seq=SEQ, d=D):
    """outs[c] is [d, bsz*tsb]; column block b holds tokens
    b*seq + [c*tsb, (c+1)*tsb) for batches 0..2; batch 3 is exchanged in
    halves: cols 3*tsb + h*tsh + [0, tsh) hold tokens
    3*seq + h*seq/2 + [c*tsh, (c+1)*tsh)."""
    tsb = seq // n_cores
    tsh = tsb // 2
    T = bsz * seq
    outT = np.empty((d, T), np.float32)
    for c in range(n_cores):
        for b in range(bsz):
            if b == bsz - 1:
                for st, ln in ((0, seq // 2), (seq // 2, seq // 2)):
                    w = ln // n_cores
                    base = b * seq + st
                    co = b * tsb + st // n_cores
                    outT[:, base + c * w:base + (c + 1) * w] = \
                        outs[c][:, co:co + w]
            else:
                outT[:, b * seq + c * tsb:b * seq + (c + 1) * tsb] = \
                    outs[c][:, b * tsb:(b + 1) * tsb]
    return np.ascontiguousarray(outT.T).reshape(bsz, seq, d)


_NC_CACHE = {}


def kernel(x, w_qkv, w_proj):
    key = "full"
    if key not in _NC_CACHE:
        _NC_CACHE[key] = build_nc()
    nc = _NC_CACHE[key]
    in_maps = shard_inputs(x, w_qkv, w_proj)
    res = run_bass_kernel_spmd(nc, in_maps, list(range(N_CORES))).results
    return assemble_out([res[c]["out"] for c in range(N_CORES)]).astype(np.float32)
